# revision 12
# baseline (speedup 1.0000x reference)
"""Causal self-attention (B=2, T=2048, D=1024, H=16) on 8 trn2 cores.

Sharding: tensor-parallel over heads x data-parallel over batch.
Core c handles batch b = c // 4, head group g = c % 4 (heads 4g..4g+3).
Host pre-slices/pre-transposes weight+activation shards; each core
returns a partial y (its heads' contribution); host sums groups of 4.

All matmul operands are bf16 (fp32 psum accumulation) — f32r matmuls
lower to fp32_mode=HIGH and trip the PE's 50%-utilization DVFS
throttle, roughly doubling stream time.

The attention loop is software-pipelined: the PE queue stays two
S-units ahead of the PV consumers so exp (Scalar) and causal-mask
multiply (Vector, static 0/1 mask tiles) hide under S/PV streams.
Softmax normalization is per-tile: reciprocal of the PV ones-row,
partition-broadcast on GpSimd, one fused psum-read multiply on Vector.
"""

import os
import sys

for _p in ("/opt/trn_rl_repo", "/root/.axon_site/_ro/trn_rl_repo"):
    if os.path.isdir(_p) and _p not in sys.path:
        sys.path.insert(0, _p)

import ml_dtypes
import numpy as np

import concourse.bass as bass
import concourse.mybir as mybir
import concourse.tile as tile
from concourse import bacc
from concourse.bass_utils import run_bass_kernel_spmd

F32 = mybir.dt.float32
BF16 = mybir.dt.bfloat16

B, T, C = 2, 2048, 1024
NHEAD_TOT = 16
DH = 64
NCORES = 8
NH = 4          # heads per core
NPAIR = 2       # head pairs per core
CK = C // 128   # contraction chunks (8)
TT = 512        # attention t-tile width
NTT = T // TT   # 4
NSCH = T // 128  # s chunks (16)
FQK = 2 * NH * DH  # 512 cols of qkv^T for q+k
FV = NH * DH       # 256 cols for v
LOOKAHEAD = int(os.environ.get("BASS_LOOKAHEAD", "2"))  # S-units in flight ahead of PV


def build_nc(dbg=False):
    nc = bacc.Bacc("TRN2", target_bir_lowering=False, debug=False)

    xT = nc.dram_tensor("xT", [C, T], BF16, kind="ExternalInput")
    wqkvT = nc.dram_tensor("wqkvT", [C, FQK + FV], BF16, kind="ExternalInput")
    woutT = nc.dram_tensor("woutT", [NH * DH, C], BF16, kind="ExternalInput")
    y = nc.dram_tensor("y", [T, C], F32, kind="ExternalOutput")
    L_dram = nc.dram_tensor("L_scratch", [2 * NPAIR * NTT, TT], F32)
    R_dram = nc.dram_tensor("R_scratch", [2 * NPAIR * NTT, TT], mybir.dt.float32r)

    EXP = mybir.ActivationFunctionType.Exp

    with tile.TileContext(nc) as tc:
        with (
            tc.tile_pool(name="const", bufs=1) as const,
            tc.tile_pool(name="ptp", bufs=4) as ptp,
            tc.tile_pool(name="bcp", bufs=2) as bcp,
            tc.tile_pool(name="rcp", bufs=2) as rcp,
            tc.tile_pool(name="yp", bufs=2) as yp,
            tc.tile_pool(name="psA", bufs=2, space="PSUM") as psA,
            tc.tile_pool(name="psV", bufs=4, space="PSUM") as psV,
        ):
            # ---- persistent SBUF ----
            xT_sb = const.tile([128, CK, T], BF16)          # x^T  (c-major)
            wqkvT_sb = const.tile([128, CK, FQK + FV], BF16)  # W_qkv^T cols [q(4x64)|k(4x64)|v(4x64)]
            woutT_sb = const.tile([128, NPAIR, C], BF16)    # W_out^T rows per head pair
            qkT_sb = const.tile([128, 4, T], BF16)          # [qPair0|qPair1|kPair0|kPair1] x T
            v_sb = const.tile([128, NSCH, NH, DH + 1], BF16)  # V (s-major) + ones column
            oT_sb = const.tile([128, NPAIR, T], BF16)       # unnormalized O^T, pair-stacked

            for ci in range(CK):
                nc.sync.dma_start(xT_sb[:, ci, :], xT[ci * 128:(ci + 1) * 128, :])
                nc.sync.dma_start(wqkvT_sb[:, ci, :], wqkvT[ci * 128:(ci + 1) * 128, :])
            for pr in range(NPAIR):
                nc.sync.dma_start(woutT_sb[:, pr, :], woutT[pr * 128:(pr + 1) * 128, :])
            # 1.0 bit pattern for the bf16 ones column
            nc.vector.memset(v_sb[:, :, :, DH:DH + 1].bitcast(mybir.dt.uint16),
                             0x3F80)
            ones1 = const.tile([1, 64], mybir.dt.float32r)  # lhsT for recipL broadcast
            nc.vector.memset(ones1.bitcast(mybir.dt.uint32), 0x3F800000)

            # ---- QKV projection ----
            # q^T/k^T: psum[f128, t512] = sum_c wqkvT[c, f].T @ xT[c, t]
            for ft in range(4):
                for tt in range(NTT):
                    ps = psA.tile([128, 2, TT], F32)
                    for ci in range(CK):
                        nc.tensor.matmul(
                            ps[:, 0, :],
                            wqkvT_sb[:, ci, ft * 128:(ft + 1) * 128],
                            xT_sb[:, ci, tt * TT:(tt + 1) * TT],
                            start=(ci == 0), stop=(ci == CK - 1),
                        )
                    nc.vector.tensor_copy(qkT_sb[:, ft, tt * TT:(tt + 1) * TT], ps[:, 0, :])
            # v natural: psum[t128, f256] = xT[c, t].T @ wqkvT[c, v]
            for si in range(NSCH):
                ps = psA.tile([128, 2, TT], F32)
                for ci in range(CK):
                    nc.tensor.matmul(
                        ps[:, 0, 0:FV],
                        xT_sb[:, ci, si * 128:(si + 1) * 128],
                        wqkvT_sb[:, ci, FQK:FQK + FV],
                        start=(ci == 0), stop=(ci == CK - 1),
                    )
                nc.vector.tensor_copy(
                    v_sb[:, si, :, 0:DH],
                    ps[:, 0, 0:FV].rearrange("p (h d) -> p h d", h=NH),
                )

            # ---- attention (S^T orientation), per head pair ----
            # software-pipelined: PE queue runs S(u+2) before PV(u) so the
            # exp+mask latency hides under two units of PE stream time.
            for pr in range(NPAIR):
                for tt in range(NTT):
                    n_ss = 4 * (tt + 1)  # causal: s-chunks 0 .. 4*tt+3
                    units = [(sq, hi) for sq in range(n_ss // 2) for hi in range(2)]
                    pv = [psV.tile([DH + 1, TT], F32, tag="pv", name=f"pv{pr}_{tt}_{k}")
                          for k in range(2)]
                    pts = {}

                    def emit_S(u, pr=pr, tt=tt, pts=pts):
                        sq, hi = units[u]
                        ps = psA.tile([128, 2, TT], F32)
                        for i in range(2):
                            ss = 2 * sq + i
                            nc.tensor.matmul(
                                ps[:, i, :],
                                qkT_sb[hi * 64:(hi + 1) * 64, 2 + pr, ss * 128:(ss + 1) * 128],
                                qkT_sb[hi * 64:(hi + 1) * 64, pr, tt * TT:(tt + 1) * TT],
                            )
                        pt = ptp.tile([128, 2, TT], BF16)
                        nc.scalar.activation(pt, ps, EXP, scale=0.125)
                        if sq >= 2 * tt:  # diagonal quad: zero where s > t
                            nc.gpsimd.affine_select(
                                out=pt, in_=pt,
                                compare_op=mybir.AluOpType.is_ge,
                                fill=0.0,
                                base=tt * TT - 2 * sq * 128,
                                channel_multiplier=-1,
                                pattern=[[-128, 2], [1, TT]],
                            )
                        pts[u] = pt

                    def emit_PV(u, pr=pr, tt=tt, n_ss=n_ss, pv=pv, pts=pts):
                        sq, hi = units[u]
                        h = pr * 2 + hi
                        pt = pts.pop(u)
                        for i in range(2):
                            ss = 2 * sq + i
                            nc.tensor.matmul(
                                pv[hi],
                                v_sb[:, ss, h, :],
                                pt[:, i, :],
                                start=(ss == 0), stop=(ss == n_ss - 1),
                            )

                    n_units = len(units)
                    for u in range(min(LOOKAHEAD, n_units)):
                        emit_S(u)
                    for u in range(n_units):
                        if u + LOOKAHEAD < n_units:
                            emit_S(u + LOOKAHEAD)
                        emit_PV(u)

                    for hi in range(2):
                        idx = (pr * NTT + tt) * 2 + hi
                        nc.vector.tensor_copy(
                            oT_sb[hi * 64:(hi + 1) * 64, pr, tt * TT:(tt + 1) * TT],
                            pv[hi][0:DH, :],
                        )
                        lrow = rcp.tile([1, TT], F32, tag="lrow", name=f"lrow{idx}")
                        nc.vector.tensor_copy(lrow, pv[hi][DH:DH + 1, :])
                        nc.sync.dma_start(L_dram[idx:idx + 1, :], lrow[0:1, :])

            # ---- batched softmax normalization ----
            # gather all 16 L rows as [128, 64], one fast reciprocal, ship
            # back, then per-tile outer-product broadcast + multiply.
            lsq = bcp.tile([128, 64], F32, tag="lsq")
            nc.sync.dma_start(lsq, L_dram[:, :].rearrange("r (s j) -> (r s) j", j=64))
            with nc.allow_low_precision("f32r recip feeds f32r matmul rhs"):
                rsq = bcp.tile([128, 64], mybir.dt.float32r, tag="rsq")
                nc.vector.reciprocal(rsq, lsq)
            nc.sync.dma_start(R_dram[:, :].rearrange("r (s j) -> (r s) j", j=64), rsq)
            for pr in range(NPAIR):
                for tt in range(NTT):
                    bq = psA.tile([128, 2, TT], F32, tag="ps", name=f"bq{pr}_{tt}")
                    bc = bcp.tile([128, TT], BF16)
                    for hi in range(2):
                        idx = (pr * NTT + tt) * 2 + hi
                        rcr = rcp.tile([1, TT], mybir.dt.float32r, tag="rcr", name=f"rcr{idx}")
                        nc.sync.dma_start(rcr[0:1, :], R_dram[idx:idx + 1, :])
                        nc.tensor.matmul(bq[:, hi, :][0:64, :], ones1, rcr)
                        nc.vector.tensor_copy(
                            bc[hi * 64:(hi + 1) * 64, :], bq[:, hi, :][0:64, :]
                        )
                        nc.vector.tensor_mul(
                            oT_sb[hi * 64:(hi + 1) * 64, pr, tt * TT:(tt + 1) * TT],
                            oT_sb[hi * 64:(hi + 1) * 64, pr, tt * TT:(tt + 1) * TT],
                            bc[hi * 64:(hi + 1) * 64, :],
                        )

            # ---- output projection: y[t, o] = sum_pr oT[d, t].T @ woutT[d, o] ----
            for tq in range(T // 128):
                for ot in range(C // TT):
                    ps = psA.tile([128, 2, TT], F32)
                    for pr in range(NPAIR):
                        nc.tensor.matmul(
                            ps[:, 0, :],
                            oT_sb[:, pr, tq * 128:(tq + 1) * 128],
                            woutT_sb[:, pr, ot * TT:(ot + 1) * TT],
                            start=(pr == 0), stop=(pr == NPAIR - 1),
                        )
                    yt = yp.tile([128, TT], F32)
                    nc.vector.tensor_copy(yt, ps[:, 0, :])
                    nc.sync.dma_start(y[tq * 128:(tq + 1) * 128, ot * TT:(ot + 1) * TT], yt)

    nc.compile()
    return nc


_NC_CACHE = None


def _get_nc():
    global _NC_CACHE
    if _NC_CACHE is None:
        _NC_CACHE = build_nc()
    return _NC_CACHE


def make_in_maps(x, W_qkv, W_out):
    x = np.ascontiguousarray(np.asarray(x, dtype=np.float32))
    W_qkv = np.ascontiguousarray(np.asarray(W_qkv, dtype=np.float32))
    W_out = np.ascontiguousarray(np.asarray(W_out, dtype=np.float32))
    bf16 = ml_dtypes.bfloat16
    xT = [np.ascontiguousarray(x[b].T.astype(bf16)) for b in range(B)]
    in_maps = []
    for c in range(NCORES):
        b, g = c // 4, c % 4
        rq = W_qkv[g * 256:(g + 1) * 256]            # q rows, heads 4g..4g+3
        rk = W_qkv[C + g * 256:C + (g + 1) * 256]    # k rows
        rv = W_qkv[2 * C + g * 256:2 * C + (g + 1) * 256]  # v rows
        wqkvT = np.ascontiguousarray(
            np.concatenate([rq, rk, rv], axis=0).T.astype(bf16))
        woutT = np.ascontiguousarray(
            W_out[:, g * 256:(g + 1) * 256].T.astype(bf16))
        in_maps.append({"xT": xT[b], "wqkvT": wqkvT, "woutT": woutT})
    return in_maps


def kernel(x, W_qkv, W_out):
    nc = _get_nc()
    in_maps = make_in_maps(x, W_qkv, W_out)
    res = run_bass_kernel_spmd(nc, in_maps, core_ids=list(range(NCORES)))
    kernel.last_results = res
    y = np.zeros((B, T, C), dtype=np.float32)
    for c in range(NCORES):
        y[c // 4] += res.results[c]["y"]
    return y


# revision 19
# speedup vs baseline: 1.0992x; 1.0992x over previous
"""Causal self-attention (B=2, T=2048, D=1024, H=16) on 8 trn2 cores.

Sharding: tensor-parallel over heads x data-parallel over batch.
Core c handles batch b = c // 4, head group g = c % 4 (heads 4g..4g+3).
Host pre-slices/pre-transposes weight+activation shards; each core
returns a partial y (its heads' contribution); host sums groups of 4.

All matmul operands are bf16 (fp32 psum accumulation) — f32r matmuls
lower to fp32_mode=HIGH and trip the PE's 50%-utilization DVFS
throttle, roughly doubling stream time.

The attention loop is software-pipelined: the PE queue stays two
S-units ahead of the PV consumers so exp (Scalar) and causal-mask
multiply (Vector, static 0/1 mask tiles) hide under S/PV streams.
Softmax normalization is per-tile: reciprocal of the PV ones-row,
partition-broadcast on GpSimd, one fused psum-read multiply on Vector.
"""

import os
import sys

for _p in ("/opt/trn_rl_repo", "/root/.axon_site/_ro/trn_rl_repo"):
    if os.path.isdir(_p) and _p not in sys.path:
        sys.path.insert(0, _p)

import ml_dtypes
import numpy as np

import concourse.bass as bass
import concourse.mybir as mybir
import concourse.tile as tile
from concourse import bacc
from concourse.bass_utils import run_bass_kernel_spmd

F32 = mybir.dt.float32
BF16 = mybir.dt.bfloat16

B, T, C = 2, 2048, 1024
NHEAD_TOT = 16
DH = 64
NCORES = 8
NH = 4          # heads per core
NPAIR = 2       # head pairs per core
CK = C // 128   # contraction chunks (8)
TT = 512        # attention t-tile width
NTT = T // TT   # 4
NSCH = T // 128  # s chunks (16)
FQK = 2 * NH * DH  # 512 cols of qkv^T for q+k
FV = NH * DH       # 256 cols for v
LOOKAHEAD = int(os.environ.get("BASS_LOOKAHEAD", "2"))  # S-units in flight ahead of PV


def build_nc(dbg=False):
    nc = bacc.Bacc("TRN2", target_bir_lowering=False, debug=False)

    xT = nc.dram_tensor("xT", [C, T], BF16, kind="ExternalInput")
    wqkvT = nc.dram_tensor("wqkvT", [C, FQK + FV], BF16, kind="ExternalInput")
    woutT = nc.dram_tensor("woutT", [NH * DH, C], BF16, kind="ExternalInput")
    y = nc.dram_tensor("y", [T, C], F32, kind="ExternalOutput")
    L_dram = nc.dram_tensor("L_scratch", [2 * NPAIR * NTT, TT], F32)
    R_dram = nc.dram_tensor("R_scratch", [2 * NPAIR * NTT, TT], BF16)

    EXP = mybir.ActivationFunctionType.Exp

    with tile.TileContext(nc) as tc:
        with (
            tc.tile_pool(name="const", bufs=1) as const,
            tc.tile_pool(name="ptp", bufs=4) as ptp,
            tc.tile_pool(name="bcp", bufs=2) as bcp,
            tc.tile_pool(name="rcp", bufs=2) as rcp,
            tc.tile_pool(name="yp", bufs=2) as yp,
            tc.tile_pool(name="psA", bufs=2, space="PSUM") as psA,
            tc.tile_pool(name="psV", bufs=4, space="PSUM") as psV,
        ):
            # ---- persistent SBUF ----
            xT_sb = const.tile([128, CK, T], BF16)          # x^T  (c-major)
            wqkvT_sb = const.tile([128, CK, FQK + FV], BF16)  # W_qkv^T cols [q(4x64)|k(4x64)|v(4x64)]
            woutT_sb = const.tile([128, NPAIR, C], BF16)    # W_out^T rows per head pair
            qkT_sb = const.tile([128, 4, T], BF16)          # [qPair0|qPair1|kPair0|kPair1] x T
            v_sb = const.tile([128, NSCH, NH, DH + 1], BF16)  # V (s-major) + ones column
            oT_sb = const.tile([128, NPAIR, T], BF16)       # unnormalized O^T, pair-stacked

            for ci in range(CK):
                nc.sync.dma_start(xT_sb[:, ci, :], xT[ci * 128:(ci + 1) * 128, :])
                nc.sync.dma_start(wqkvT_sb[:, ci, :], wqkvT[ci * 128:(ci + 1) * 128, :])
            for pr in range(NPAIR):
                nc.sync.dma_start(woutT_sb[:, pr, :], woutT[pr * 128:(pr + 1) * 128, :])
            # 1.0 bit pattern for the bf16 ones column
            nc.vector.memset(v_sb[:, :, :, DH:DH + 1].bitcast(mybir.dt.uint16),
                             0x3F80)

            # ---- QKV projection ----
            # q^T/k^T: psum[f128, t512] = sum_c wqkvT[c, f].T @ xT[c, t]
            for ft in range(4):
                for tt in range(NTT):
                    ps = psA.tile([128, 2, TT], F32)
                    for ci in range(CK):
                        nc.tensor.matmul(
                            ps[:, 0, :],
                            wqkvT_sb[:, ci, ft * 128:(ft + 1) * 128],
                            xT_sb[:, ci, tt * TT:(tt + 1) * TT],
                            start=(ci == 0), stop=(ci == CK - 1),
                        )
                    nc.vector.tensor_copy(qkT_sb[:, ft, tt * TT:(tt + 1) * TT], ps[:, 0, :])
            # v natural: psum[t128, f256] = xT[c, t].T @ wqkvT[c, v]
            for si in range(NSCH):
                ps = psA.tile([128, 2, TT], F32)
                for ci in range(CK):
                    nc.tensor.matmul(
                        ps[:, 0, 0:FV],
                        xT_sb[:, ci, si * 128:(si + 1) * 128],
                        wqkvT_sb[:, ci, FQK:FQK + FV],
                        start=(ci == 0), stop=(ci == CK - 1),
                    )
                nc.vector.tensor_copy(
                    v_sb[:, si, :, 0:DH],
                    ps[:, 0, 0:FV].rearrange("p (h d) -> p h d", h=NH),
                )

            # ---- attention (S^T orientation), per head pair ----
            # software-pipelined: PE queue runs S(u+2) before PV(u) so the
            # exp+mask latency hides under two units of PE stream time.
            for pr in range(NPAIR):
                for tt in range(NTT):
                    n_ss = 4 * (tt + 1)  # causal: s-chunks 0 .. 4*tt+3
                    units = [(sq, hi) for sq in range(n_ss // 2) for hi in range(2)]
                    pv = [psV.tile([DH + 1, TT], F32, tag="pv", name=f"pv{pr}_{tt}_{k}")
                          for k in range(2)]
                    pts = {}

                    def emit_S(u, pr=pr, tt=tt, pts=pts):
                        sq, hi = units[u]
                        ps = psA.tile([128, 2, TT], F32)
                        for i in range(2):
                            ss = 2 * sq + i
                            nc.tensor.matmul(
                                ps[:, i, :],
                                qkT_sb[hi * 64:(hi + 1) * 64, 2 + pr, ss * 128:(ss + 1) * 128],
                                qkT_sb[hi * 64:(hi + 1) * 64, pr, tt * TT:(tt + 1) * TT],
                            )
                        pt = ptp.tile([128, 2, TT], BF16)
                        nc.scalar.activation(pt, ps, EXP, scale=0.125)
                        if sq >= 2 * tt:  # diagonal quad: zero where s > t
                            nc.gpsimd.affine_select(
                                out=pt, in_=pt,
                                compare_op=mybir.AluOpType.is_ge,
                                fill=0.0,
                                base=tt * TT - 2 * sq * 128,
                                channel_multiplier=-1,
                                pattern=[[-128, 2], [1, TT]],
                            )
                        pts[u] = pt

                    def emit_PV(u, pr=pr, tt=tt, n_ss=n_ss, pv=pv, pts=pts):
                        sq, hi = units[u]
                        h = pr * 2 + hi
                        pt = pts.pop(u)
                        for i in range(2):
                            ss = 2 * sq + i
                            nc.tensor.matmul(
                                pv[hi],
                                v_sb[:, ss, h, :],
                                pt[:, i, :],
                                start=(ss == 0), stop=(ss == n_ss - 1),
                            )

                    n_units = len(units)
                    for u in range(min(LOOKAHEAD, n_units)):
                        emit_S(u)
                    for u in range(n_units):
                        if u + LOOKAHEAD < n_units:
                            emit_S(u + LOOKAHEAD)
                        emit_PV(u)

                    for hi in range(2):
                        idx = (pr * NTT + tt) * 2 + hi
                        nc.vector.tensor_copy(
                            oT_sb[hi * 64:(hi + 1) * 64, pr, tt * TT:(tt + 1) * TT],
                            pv[hi][0:DH, :],
                        )
                        lrow = rcp.tile([1, TT], F32, tag="lrow", name=f"lrow{idx}")
                        nc.vector.tensor_copy(lrow, pv[hi][DH:DH + 1, :])
                        nc.sync.dma_start(L_dram[idx:idx + 1, :], lrow[0:1, :])

                # ---- per-pair softmax normalization ----
                # gather this pair's 8 L rows as [64, 64], one fast reciprocal,
                # ship back, then per-row partition-broadcast (GpSimd) and
                # multiply (Vector). For pr=0 this overlaps pr=1's attention.
                lsq = bcp.tile([64, 64], F32, tag="lsq", name=f"lsq{pr}")
                nc.sync.dma_start(
                    lsq, L_dram[pr * 8:(pr + 1) * 8, :].rearrange("r (s j) -> (r s) j", j=64))
                with nc.allow_low_precision("1/L in bf16; L is O(1)-scaled"):
                    rsq = bcp.tile([64, 64], BF16, tag="rsq", name=f"rsq{pr}")
                    nc.vector.reciprocal(rsq, lsq)
                nc.sync.dma_start(
                    R_dram[pr * 8:(pr + 1) * 8, :].rearrange("r (s j) -> (r s) j", j=64), rsq)
                for tt in range(NTT):
                    idx = (pr * NTT + tt) * 2
                    rcr0 = rcp.tile([1, TT], BF16, tag="rcr", name=f"rcr{idx}")
                    rcr1 = rcp.tile([1, TT], BF16, tag="rcr", name=f"rcr{idx + 1}")
                    nc.sync.dma_start(rcr0[0:1, :], R_dram[idx:idx + 1, :])
                    nc.sync.dma_start(rcr1[0:1, :], R_dram[idx + 1:idx + 2, :])
                    # [128, TT] = 1/L for hi=1 everywhere, lower half
                    # overwritten with hi=0's row -> one aligned mul
                    rb = bcp.tile([128, TT], BF16, tag="rb", name=f"rb{idx}")
                    nc.gpsimd.partition_broadcast(rb, rcr1[0:1, :], channels=128)
                    nc.gpsimd.partition_broadcast(rb[0:64, :], rcr0[0:1, :], channels=64)
                    nc.vector.tensor_mul(
                        oT_sb[:, pr, tt * TT:(tt + 1) * TT],
                        oT_sb[:, pr, tt * TT:(tt + 1) * TT],
                        rb,
                    )

            # ---- output projection: y[t, o] = sum_pr oT[d, t].T @ woutT[d, o] ----
            # psum copy-out on the Scalar engine (idle in this phase; Vector
            # would serialize behind the norm muls).
            for tq in range(T // 128):
                for ot in range(C // TT):
                    ps = psA.tile([128, 2, TT], F32)
                    for pr in range(NPAIR):
                        nc.tensor.matmul(
                            ps[:, 0, :],
                            oT_sb[:, pr, tq * 128:(tq + 1) * 128],
                            woutT_sb[:, pr, ot * TT:(ot + 1) * TT],
                            start=(pr == 0), stop=(pr == NPAIR - 1),
                        )
                    yt = yp.tile([128, TT], F32)
                    nc.scalar.copy(yt, ps[:, 0, :])
                    nc.sync.dma_start(y[tq * 128:(tq + 1) * 128, ot * TT:(ot + 1) * TT], yt)

    nc.compile()
    return nc


_NC_CACHE = None


def _get_nc():
    global _NC_CACHE
    if _NC_CACHE is None:
        _NC_CACHE = build_nc()
    return _NC_CACHE


def make_in_maps(x, W_qkv, W_out):
    x = np.ascontiguousarray(np.asarray(x, dtype=np.float32))
    W_qkv = np.ascontiguousarray(np.asarray(W_qkv, dtype=np.float32))
    W_out = np.ascontiguousarray(np.asarray(W_out, dtype=np.float32))
    bf16 = ml_dtypes.bfloat16
    xT = [np.ascontiguousarray(x[b].T.astype(bf16)) for b in range(B)]
    in_maps = []
    for c in range(NCORES):
        b, g = c // 4, c % 4
        rq = W_qkv[g * 256:(g + 1) * 256]            # q rows, heads 4g..4g+3
        rk = W_qkv[C + g * 256:C + (g + 1) * 256]    # k rows
        rv = W_qkv[2 * C + g * 256:2 * C + (g + 1) * 256]  # v rows
        wqkvT = np.ascontiguousarray(
            np.concatenate([rq, rk, rv], axis=0).T.astype(bf16))
        woutT = np.ascontiguousarray(
            W_out[:, g * 256:(g + 1) * 256].T.astype(bf16))
        in_maps.append({"xT": xT[b], "wqkvT": wqkvT, "woutT": woutT})
    return in_maps


def kernel(x, W_qkv, W_out):
    nc = _get_nc()
    in_maps = make_in_maps(x, W_qkv, W_out)
    res = run_bass_kernel_spmd(nc, in_maps, core_ids=list(range(NCORES)))
    kernel.last_results = res
    y = np.zeros((B, T, C), dtype=np.float32)
    for c in range(NCORES):
        y[c // 4] += res.results[c]["y"]
    return y


# revision 23
# speedup vs baseline: 1.3284x; 1.2086x over previous
"""Causal self-attention (B=2, T=2048, D=1024, H=16) on 8 trn2 cores.

Sharding: tensor-parallel over heads x data-parallel over batch.
Core c handles batch b = c // 4, head group g = c % 4 (heads 4g..4g+3).
Host pre-slices/pre-transposes weight+activation shards; each core
returns a partial y (its heads' contribution); host sums groups of 4.

All matmul operands are bf16 (fp32 psum accumulation) — f32r matmuls
lower to fp32_mode=HIGH and trip the PE's 50%-utilization DVFS
throttle, roughly doubling stream time.

The attention loop is software-pipelined: the PE queue stays two
S-units ahead of the PV consumers so exp (Scalar) and causal-mask
multiply (Vector, static 0/1 mask tiles) hide under S/PV streams.
Softmax normalization is per-tile: reciprocal of the PV ones-row,
partition-broadcast on GpSimd, one fused psum-read multiply on Vector.
"""

import os
import sys

for _p in ("/opt/trn_rl_repo", "/root/.axon_site/_ro/trn_rl_repo"):
    if os.path.isdir(_p) and _p not in sys.path:
        sys.path.insert(0, _p)

import ml_dtypes
import numpy as np

import concourse.bass as bass
import concourse.mybir as mybir
import concourse.tile as tile
from concourse import bacc
from concourse.bass_utils import run_bass_kernel_spmd

F32 = mybir.dt.float32
BF16 = mybir.dt.bfloat16

B, T, C = 2, 2048, 1024
NHEAD_TOT = 16
DH = 64
NCORES = 8
NH = 4          # heads per core
NPAIR = 2       # head pairs per core
CK = C // 128   # contraction chunks (8)
TT = 512        # attention t-tile width
NTT = T // TT   # 4
NSCH = T // 128  # s chunks (16)
FQK = 2 * NH * DH  # 512 cols of qkv^T for q+k
FV = NH * DH       # 256 cols for v
LOOKAHEAD = int(os.environ.get("BASS_LOOKAHEAD", "2"))  # S-units in flight ahead of PV


def build_nc(dbg=False):
    nc = bacc.Bacc("TRN2", target_bir_lowering=False, debug=False)

    xT = nc.dram_tensor("xT", [C, T], BF16, kind="ExternalInput")
    wqkvT = nc.dram_tensor("wqkvT", [C, FQK + FV], BF16, kind="ExternalInput")
    woutT = nc.dram_tensor("woutT", [NH * DH, C], BF16, kind="ExternalInput")
    y = nc.dram_tensor("y", [T, C], F32, kind="ExternalOutput")
    L_dram = nc.dram_tensor("L_scratch", [2 * NPAIR * NTT, TT], F32)
    R_dram = nc.dram_tensor("R_scratch", [2 * NPAIR * NTT, TT], BF16)

    EXP = mybir.ActivationFunctionType.Exp

    with tile.TileContext(nc) as tc:
        with (
            tc.tile_pool(name="const", bufs=1) as const,
            tc.tile_pool(name="ptp", bufs=4) as ptp,
            tc.tile_pool(name="bcp", bufs=2) as bcp,
            tc.tile_pool(name="rcp", bufs=2) as rcp,
            tc.tile_pool(name="yp", bufs=2) as yp,
            tc.tile_pool(name="psA", bufs=2, space="PSUM") as psA,
            tc.tile_pool(name="psV", bufs=4, space="PSUM") as psV,
        ):
            # ---- persistent SBUF ----
            xT_sb = const.tile([128, CK, T], BF16)          # x^T  (c-major)
            wqkvT_sb = const.tile([128, CK, FQK + FV], BF16)  # W_qkv^T cols [q(4x64)|k(4x64)|v(4x64)]
            woutT_sb = const.tile([128, NPAIR, C], BF16)    # W_out^T rows per head pair
            qkT_sb = const.tile([128, 4, T], BF16)          # [qPair0|qPair1|kPair0|kPair1] x T
            v_sb = const.tile([128, NSCH, NH, DH + 1], BF16)  # V (s-major) + ones column
            oT_sb = const.tile([128, NPAIR, T], BF16)       # unnormalized O^T, pair-stacked

            for ci in range(CK):
                nc.sync.dma_start(xT_sb[:, ci, :], xT[ci * 128:(ci + 1) * 128, :])
                nc.sync.dma_start(wqkvT_sb[:, ci, :], wqkvT[ci * 128:(ci + 1) * 128, :])
            for pr in range(NPAIR):
                nc.sync.dma_start(woutT_sb[:, pr, :], woutT[pr * 128:(pr + 1) * 128, :])
            # 1.0 bit pattern for the bf16 ones column
            nc.vector.memset(v_sb[:, :, :, DH:DH + 1].bitcast(mybir.dt.uint16),
                             0x3F80)

            # ---- QKV projection ----
            # q^T/k^T: psum[f128, t512] = sum_c wqkvT[c, f].T @ xT[c, t]
            for ft in range(4):
                for tt in range(NTT):
                    ps = psA.tile([128, 2, TT], F32)
                    for ci in range(CK):
                        nc.tensor.matmul(
                            ps[:, 0, :],
                            wqkvT_sb[:, ci, ft * 128:(ft + 1) * 128],
                            xT_sb[:, ci, tt * TT:(tt + 1) * TT],
                            start=(ci == 0), stop=(ci == CK - 1),
                        )
                    nc.vector.tensor_copy(qkT_sb[:, ft, tt * TT:(tt + 1) * TT], ps[:, 0, :])
            # v natural: psum[t128, f256] = xT[c, t].T @ wqkvT[c, v]
            for si in range(NSCH):
                ps = psA.tile([128, 2, TT], F32)
                for ci in range(CK):
                    nc.tensor.matmul(
                        ps[:, 0, 0:FV],
                        xT_sb[:, ci, si * 128:(si + 1) * 128],
                        wqkvT_sb[:, ci, FQK:FQK + FV],
                        start=(ci == 0), stop=(ci == CK - 1),
                    )
                nc.vector.tensor_copy(
                    v_sb[:, si, :, 0:DH],
                    ps[:, 0, 0:FV].rearrange("p (h d) -> p h d", h=NH),
                )

            # ---- attention (S^T orientation), per head pair ----
            # software-pipelined: PE queue runs S(u+2) before PV(u) so the
            # exp+mask latency hides under two units of PE stream time.
            for pr in range(NPAIR):
                for tt in range(NTT):
                    n_ss = 4 * (tt + 1)  # causal: s-chunks 0 .. 4*tt+3
                    units = [(sq, hi) for sq in range(n_ss // 2) for hi in range(2)]
                    pv = [psV.tile([DH + 1, TT], F32, tag="pv", name=f"pv{pr}_{tt}_{k}")
                          for k in range(2)]
                    pts = {}

                    def emit_S(u, pr=pr, tt=tt, pts=pts):
                        sq, hi = units[u]
                        ps = psA.tile([128, 2, TT], F32)
                        for i in range(2):
                            ss = 2 * sq + i
                            nc.tensor.matmul(
                                ps[:, i, :],
                                qkT_sb[hi * 64:(hi + 1) * 64, 2 + pr, ss * 128:(ss + 1) * 128],
                                qkT_sb[hi * 64:(hi + 1) * 64, pr, tt * TT:(tt + 1) * TT],
                            )
                        pt = ptp.tile([128, 2, TT], BF16)
                        nc.scalar.activation(pt, ps, EXP, scale=0.125)
                        if sq >= 2 * tt:  # diagonal quad: zero where s > t
                            nc.gpsimd.affine_select(
                                out=pt, in_=pt,
                                compare_op=mybir.AluOpType.is_ge,
                                fill=0.0,
                                base=tt * TT - 2 * sq * 128,
                                channel_multiplier=-1,
                                pattern=[[-128, 2], [1, TT]],
                            )
                        pts[u] = pt

                    def emit_PV(u, pr=pr, tt=tt, n_ss=n_ss, pv=pv, pts=pts):
                        sq, hi = units[u]
                        h = pr * 2 + hi
                        pt = pts.pop(u)
                        for i in range(2):
                            ss = 2 * sq + i
                            nc.tensor.matmul(
                                pv[hi],
                                v_sb[:, ss, h, :],
                                pt[:, i, :],
                                start=(ss == 0), stop=(ss == n_ss - 1),
                            )

                    n_units = len(units)
                    for u in range(min(LOOKAHEAD, n_units)):
                        emit_S(u)
                    for u in range(n_units):
                        if u + LOOKAHEAD < n_units:
                            emit_S(u + LOOKAHEAD)
                        emit_PV(u)

                    # ---- per-tile softmax normalization ----
                    # L rows reshaped to a 16-partition layout via SBUF->SBUF
                    # DMA (single-partition reciprocal is ~6x slower), one
                    # reciprocal, reshaped back, partition-broadcast on GpSimd,
                    # one aligned [128, TT] multiply on Vector. For tt < last
                    # this all hides under the next tile's attention stream.
                    lg = rcp.tile([16, 64], F32, tag="lg", name=f"lg{pr}_{tt}")
                    for hi in range(2):
                        idx = (pr * NTT + tt) * 2 + hi
                        nc.vector.tensor_copy(
                            oT_sb[hi * 64:(hi + 1) * 64, pr, tt * TT:(tt + 1) * TT],
                            pv[hi][0:DH, :],
                        )
                        lrow = rcp.tile([1, TT], F32, tag="lrow", name=f"lrow{idx}")
                        nc.vector.tensor_copy(lrow, pv[hi][DH:DH + 1, :])
                        nc.sync.dma_start(L_dram[idx:idx + 1, :], lrow[0:1, :])
                    base = (pr * NTT + tt) * 2
                    nc.sync.dma_start(
                        lg, L_dram[base:base + 2, :].rearrange("r (s j) -> (r s) j", j=64))
                    with nc.allow_low_precision("1/L in bf16; L is O(1)-scaled"):
                        rc = rcp.tile([16, 64], BF16, tag="rc", name=f"rc{pr}_{tt}")
                        nc.vector.reciprocal(rc, lg)
                    nc.sync.dma_start(
                        R_dram[base:base + 2, :].rearrange("r (s j) -> (r s) j", j=64), rc)
                    rcb = []
                    for hi in range(2):
                        idx = base + hi
                        r1 = rcp.tile([1, TT], BF16, tag="rcb", name=f"rcb{pr}_{tt}_{hi}")
                        nc.sync.dma_start(r1[0:1, :], R_dram[idx:idx + 1, :])
                        rcb.append(r1)
                    rb = bcp.tile([128, TT], BF16, tag="rb", name=f"rb{pr}_{tt}")
                    nc.gpsimd.partition_broadcast(rb, rcb[1][0:1, :], channels=128)
                    nc.gpsimd.partition_broadcast(rb[0:64, :], rcb[0][0:1, :], channels=64)
                    nc.vector.tensor_mul(
                        oT_sb[:, pr, tt * TT:(tt + 1) * TT],
                        oT_sb[:, pr, tt * TT:(tt + 1) * TT],
                        rb,
                    )

            # ---- output projection: y[t, o] = sum_pr oT[d, t].T @ woutT[d, o] ----
            # both psum halves per tq; psum copy-out on the Scalar engine
            # (idle in this phase), one DMA per 128-row block of y.
            for tq in range(T // 128):
                ps = psA.tile([128, 2, TT], F32)
                for ot in range(C // TT):
                    for pr in range(NPAIR):
                        nc.tensor.matmul(
                            ps[:, ot, :],
                            oT_sb[:, pr, tq * 128:(tq + 1) * 128],
                            woutT_sb[:, pr, ot * TT:(ot + 1) * TT],
                            start=(pr == 0), stop=(pr == NPAIR - 1),
                        )
                yt = yp.tile([128, 2, TT], F32)
                nc.scalar.copy(yt, ps)
                nc.sync.dma_start(
                    y[tq * 128:(tq + 1) * 128, :],
                    yt.rearrange("p a b -> p (a b)"))

    nc.compile()
    return nc


_NC_CACHE = None


def _get_nc():
    global _NC_CACHE
    if _NC_CACHE is None:
        _NC_CACHE = build_nc()
    return _NC_CACHE


def make_in_maps(x, W_qkv, W_out):
    x = np.ascontiguousarray(np.asarray(x, dtype=np.float32))
    W_qkv = np.ascontiguousarray(np.asarray(W_qkv, dtype=np.float32))
    W_out = np.ascontiguousarray(np.asarray(W_out, dtype=np.float32))
    bf16 = ml_dtypes.bfloat16
    xT = [np.ascontiguousarray(x[b].T.astype(bf16)) for b in range(B)]
    in_maps = []
    for c in range(NCORES):
        b, g = c // 4, c % 4
        rq = W_qkv[g * 256:(g + 1) * 256]            # q rows, heads 4g..4g+3
        rk = W_qkv[C + g * 256:C + (g + 1) * 256]    # k rows
        rv = W_qkv[2 * C + g * 256:2 * C + (g + 1) * 256]  # v rows
        wqkvT = np.ascontiguousarray(
            np.concatenate([rq, rk, rv], axis=0).T.astype(bf16))
        woutT = np.ascontiguousarray(
            W_out[:, g * 256:(g + 1) * 256].T.astype(bf16))
        in_maps.append({"xT": xT[b], "wqkvT": wqkvT, "woutT": woutT})
    return in_maps


def kernel(x, W_qkv, W_out):
    nc = _get_nc()
    in_maps = make_in_maps(x, W_qkv, W_out)
    res = run_bass_kernel_spmd(nc, in_maps, core_ids=list(range(NCORES)))
    kernel.last_results = res
    y = np.zeros((B, T, C), dtype=np.float32)
    for c in range(NCORES):
        y[c // 4] += res.results[c]["y"]
    return y


# revision 30
# speedup vs baseline: 1.3722x; 1.0330x over previous
"""Causal self-attention (B=2, T=2048, D=1024, H=16) on 8 trn2 cores.

Sharding: tensor-parallel over heads x data-parallel over batch.
Core c handles batch b = c // 4, head group g = c % 4 (heads 4g..4g+3).
Host pre-slices/pre-transposes weight+activation shards; each core
returns a partial y (its heads' contribution); host sums groups of 4.

All matmul operands are bf16 (fp32 psum accumulation) — f32r matmuls
lower to fp32_mode=HIGH and trip the PE's 50%-utilization DVFS
throttle, roughly doubling stream time.

The attention loop is software-pipelined: the PE queue stays two
S-units ahead of the PV consumers so exp (Scalar) and causal-mask
multiply (Vector, static 0/1 mask tiles) hide under S/PV streams.
Softmax normalization is per-tile: reciprocal of the PV ones-row,
partition-broadcast on GpSimd, one fused psum-read multiply on Vector.
"""

import os
import sys

for _p in ("/opt/trn_rl_repo", "/root/.axon_site/_ro/trn_rl_repo"):
    if os.path.isdir(_p) and _p not in sys.path:
        sys.path.insert(0, _p)

import ml_dtypes
import numpy as np

import concourse.bass as bass
import concourse.mybir as mybir
import concourse.tile as tile
from concourse import bacc
from concourse.bass_utils import run_bass_kernel_spmd

F32 = mybir.dt.float32
BF16 = mybir.dt.bfloat16

B, T, C = 2, 2048, 1024
NHEAD_TOT = 16
DH = 64
NCORES = 8
NH = 4          # heads per core
NPAIR = 2       # head pairs per core
CK = C // 128   # contraction chunks (8)
TT = 512        # attention t-tile width
NTT = T // TT   # 4
NSCH = T // 128  # s chunks (16)
FQK = 2 * NH * DH  # 512 cols of qkv^T for q+k
FV = NH * DH       # 256 cols for v
LOOKAHEAD = int(os.environ.get("BASS_LOOKAHEAD", "2"))  # S-units in flight ahead of PV


def build_nc(dbg=False):
    nc = bacc.Bacc("TRN2", target_bir_lowering=False, debug=False)

    xT = nc.dram_tensor("xT", [C, T], BF16, kind="ExternalInput")
    wqkvT = nc.dram_tensor("wqkvT", [C, FQK + FV], BF16, kind="ExternalInput")
    woutT = nc.dram_tensor("woutT", [NH * DH, C], BF16, kind="ExternalInput")
    y = nc.dram_tensor("y", [T, C], BF16, kind="ExternalOutput")
    L_dram = nc.dram_tensor("L_scratch", [2 * NPAIR * NTT, TT], F32)
    R_dram = nc.dram_tensor("R_scratch", [2 * NPAIR * NTT, TT], BF16)

    EXP = mybir.ActivationFunctionType.Exp

    with tile.TileContext(nc) as tc:
        with (
            tc.tile_pool(name="const", bufs=1) as const,
            tc.tile_pool(name="ptp", bufs=4) as ptp,
            tc.tile_pool(name="bcp", bufs=2) as bcp,
            tc.tile_pool(name="rcp", bufs=2) as rcp,
            tc.tile_pool(name="yp", bufs=4) as yp,
            tc.tile_pool(name="psA", bufs=2, space="PSUM") as psA,
            tc.tile_pool(name="psV", bufs=4, space="PSUM") as psV,
        ):
            # ---- persistent SBUF ----
            xT_sb = const.tile([128, CK, T], BF16)          # x^T  (c-major)
            wqkvT_sb = const.tile([128, CK, FQK + FV], BF16)  # W_qkv^T cols [q(4x64)|k(4x64)|v(4x64)]
            woutT_sb = const.tile([128, NPAIR, C], BF16)    # W_out^T rows per head pair
            qkT_sb = const.tile([128, 4, T], BF16)          # [qPair0|qPair1|kPair0|kPair1] x T
            v_sb = const.tile([128, NSCH, NH, DH + 1], BF16)  # V (s-major) + ones column
            oT_sb = const.tile([128, NPAIR, T], BF16)       # unnormalized O^T, pair-stacked

            # weights first, then x in t-slice-major order so the first q/k
            # groups (tt=0) start after ~1/4 of the x bytes have landed
            for ci in range(CK):
                nc.sync.dma_start(wqkvT_sb[:, ci, :], wqkvT[ci * 128:(ci + 1) * 128, :])
            for tt in range(NTT):
                for ci in range(CK):
                    nc.sync.dma_start(
                        xT_sb[:, ci, tt * TT:(tt + 1) * TT],
                        xT[ci * 128:(ci + 1) * 128, tt * TT:(tt + 1) * TT])
            for pr in range(NPAIR):
                nc.sync.dma_start(woutT_sb[:, pr, :], woutT[pr * 128:(pr + 1) * 128, :])
            # 1.0 bit pattern for the bf16 ones column
            nc.vector.memset(v_sb[:, :, :, DH:DH + 1].bitcast(mybir.dt.uint16),
                             0x3F80)

            # ---- QKV projection ----
            # q^T/k^T: psum[f128, t512] = sum_c wqkvT[c, f].T @ xT[c, t]
            # tt outer: matches the x DMA arrival order
            for tt in range(NTT):
                for ft in range(4):
                    ps = psA.tile([128, 2, TT], F32)
                    for ci in range(CK):
                        nc.tensor.matmul(
                            ps[:, 0, :],
                            wqkvT_sb[:, ci, ft * 128:(ft + 1) * 128],
                            xT_sb[:, ci, tt * TT:(tt + 1) * TT],
                            start=(ci == 0), stop=(ci == CK - 1),
                        )
                    nc.vector.tensor_copy(qkT_sb[:, ft, tt * TT:(tt + 1) * TT], ps[:, 0, :])
            # v natural: psum[t128, f256] = xT[c, t].T @ wqkvT[c, v]
            for si in range(NSCH):
                ps = psA.tile([128, 2, TT], F32)
                for ci in range(CK):
                    nc.tensor.matmul(
                        ps[:, 0, 0:FV],
                        xT_sb[:, ci, si * 128:(si + 1) * 128],
                        wqkvT_sb[:, ci, FQK:FQK + FV],
                        start=(ci == 0), stop=(ci == CK - 1),
                    )
                nc.vector.tensor_copy(
                    v_sb[:, si, :, 0:DH],
                    ps[:, 0, 0:FV].rearrange("p (h d) -> p h d", h=NH),
                )

            # ---- attention (S^T orientation), per head pair ----
            # software-pipelined: PE queue runs S(u+2) before PV(u) so the
            # exp+mask latency hides under two units of PE stream time.
            for pr in range(NPAIR):
                for tt in range(NTT):
                    n_ss = 4 * (tt + 1)  # causal: s-chunks 0 .. 4*tt+3
                    units = [(sq, hi) for sq in range(n_ss // 2) for hi in range(2)]
                    pv = [psV.tile([DH + 1, TT], F32, tag="pv", name=f"pv{pr}_{tt}_{k}")
                          for k in range(2)]
                    pts = {}

                    def emit_S(u, pr=pr, tt=tt, pts=pts):
                        sq, hi = units[u]
                        ps = psA.tile([128, 2, TT], F32)
                        for i in range(2):
                            ss = 2 * sq + i
                            nc.tensor.matmul(
                                ps[:, i, :],
                                qkT_sb[hi * 64:(hi + 1) * 64, 2 + pr, ss * 128:(ss + 1) * 128],
                                qkT_sb[hi * 64:(hi + 1) * 64, pr, tt * TT:(tt + 1) * TT],
                            )
                        pt = ptp.tile([128, 2, TT], BF16)
                        nc.scalar.activation(pt, ps, EXP, scale=0.125)
                        if sq >= 2 * tt:  # diagonal quad: zero where s > t
                            nc.gpsimd.affine_select(
                                out=pt, in_=pt,
                                compare_op=mybir.AluOpType.is_ge,
                                fill=0.0,
                                base=tt * TT - 2 * sq * 128,
                                channel_multiplier=-1,
                                pattern=[[-128, 2], [1, TT]],
                            )
                        pts[u] = pt

                    def emit_PV(u, pr=pr, tt=tt, n_ss=n_ss, pv=pv, pts=pts):
                        sq, hi = units[u]
                        h = pr * 2 + hi
                        pt = pts.pop(u)
                        for i in range(2):
                            ss = 2 * sq + i
                            nc.tensor.matmul(
                                pv[hi],
                                v_sb[:, ss, h, :],
                                pt[:, i, :],
                                start=(ss == 0), stop=(ss == n_ss - 1),
                            )

                    n_units = len(units)
                    for u in range(min(LOOKAHEAD, n_units)):
                        emit_S(u)
                    for u in range(n_units):
                        if u + LOOKAHEAD < n_units:
                            emit_S(u + LOOKAHEAD)
                        emit_PV(u)

                    # ---- per-tile softmax normalization ----
                    # L rows reshaped to a 16-partition layout via SBUF->SBUF
                    # DMA (single-partition reciprocal is ~6x slower), one
                    # reciprocal, reshaped back, partition-broadcast on GpSimd,
                    # one aligned [128, TT] multiply on Vector. For tt < last
                    # this all hides under the next tile's attention stream.
                    lg = rcp.tile([16, 64], F32, tag="lg", name=f"lg{pr}_{tt}")
                    for hi in range(2):
                        idx = (pr * NTT + tt) * 2 + hi
                        nc.vector.tensor_copy(
                            oT_sb[hi * 64:(hi + 1) * 64, pr, tt * TT:(tt + 1) * TT],
                            pv[hi][0:DH, :],
                        )
                        lrow = rcp.tile([1, TT], F32, tag="lrow", name=f"lrow{idx}")
                        nc.vector.tensor_copy(lrow, pv[hi][DH:DH + 1, :])
                        nc.sync.dma_start(L_dram[idx:idx + 1, :], lrow[0:1, :])
                    base = (pr * NTT + tt) * 2
                    nc.sync.dma_start(
                        lg, L_dram[base:base + 2, :].rearrange("r (s j) -> (r s) j", j=64))
                    with nc.allow_low_precision("1/L in bf16; L is O(1)-scaled"):
                        rc = rcp.tile([16, 64], BF16, tag="rc", name=f"rc{pr}_{tt}")
                        nc.vector.reciprocal(rc, lg)
                    nc.sync.dma_start(
                        R_dram[base:base + 2, :].rearrange("r (s j) -> (r s) j", j=64), rc)
                    rcb = []
                    for hi in range(2):
                        idx = base + hi
                        r1 = rcp.tile([1, TT], BF16, tag="rcb", name=f"rcb{pr}_{tt}_{hi}")
                        nc.sync.dma_start(r1[0:1, :], R_dram[idx:idx + 1, :])
                        rcb.append(r1)
                    rb = bcp.tile([128, TT], BF16, tag="rb", name=f"rb{pr}_{tt}")
                    nc.gpsimd.partition_broadcast(rb, rcb[1][0:1, :], channels=128)
                    nc.gpsimd.partition_broadcast(rb[0:64, :], rcb[0][0:1, :], channels=64)
                    nc.vector.tensor_mul(
                        oT_sb[:, pr, tt * TT:(tt + 1) * TT],
                        oT_sb[:, pr, tt * TT:(tt + 1) * TT],
                        rb,
                    )

            # ---- output projection: y[t, o] = sum_pr oT[d, t].T @ woutT[d, o] ----
            # both psum halves per tq; psum copy-out on the Scalar engine
            # (idle in this phase), one DMA per 128-row block of y.
            for tq in range(T // 128):
                ps = psA.tile([128, 2, TT], F32)
                for ot in range(C // TT):
                    for pr in range(NPAIR):
                        nc.tensor.matmul(
                            ps[:, ot, :],
                            oT_sb[:, pr, tq * 128:(tq + 1) * 128],
                            woutT_sb[:, pr, ot * TT:(ot + 1) * TT],
                            start=(pr == 0), stop=(pr == NPAIR - 1),
                        )
                    yt = yp.tile([128, TT], BF16)
                    nc.scalar.copy(yt, ps[:, ot, :])
                    nc.sync.dma_start(
                        y[tq * 128:(tq + 1) * 128, ot * TT:(ot + 1) * TT], yt)

    nc.compile()
    return nc


_NC_CACHE = None


def _get_nc():
    global _NC_CACHE
    if _NC_CACHE is None:
        _NC_CACHE = build_nc()
    return _NC_CACHE


def make_in_maps(x, W_qkv, W_out):
    x = np.ascontiguousarray(np.asarray(x, dtype=np.float32))
    W_qkv = np.ascontiguousarray(np.asarray(W_qkv, dtype=np.float32))
    W_out = np.ascontiguousarray(np.asarray(W_out, dtype=np.float32))
    bf16 = ml_dtypes.bfloat16
    xT = [np.ascontiguousarray(x[b].T.astype(bf16)) for b in range(B)]
    in_maps = []
    for c in range(NCORES):
        b, g = c // 4, c % 4
        rq = W_qkv[g * 256:(g + 1) * 256]            # q rows, heads 4g..4g+3
        rk = W_qkv[C + g * 256:C + (g + 1) * 256]    # k rows
        rv = W_qkv[2 * C + g * 256:2 * C + (g + 1) * 256]  # v rows
        wqkvT = np.ascontiguousarray(
            np.concatenate([rq, rk, rv], axis=0).T.astype(bf16))
        woutT = np.ascontiguousarray(
            W_out[:, g * 256:(g + 1) * 256].T.astype(bf16))
        in_maps.append({"xT": xT[b], "wqkvT": wqkvT, "woutT": woutT})
    return in_maps


def kernel(x, W_qkv, W_out):
    nc = _get_nc()
    in_maps = make_in_maps(x, W_qkv, W_out)
    res = run_bass_kernel_spmd(nc, in_maps, core_ids=list(range(NCORES)))
    kernel.last_results = res
    y = np.zeros((B, T, C), dtype=np.float32)
    for c in range(NCORES):
        y[c // 4] += res.results[c]["y"].astype(np.float32)
    return y


# revision 31
# speedup vs baseline: 1.3903x; 1.0132x over previous
"""Causal self-attention (B=2, T=2048, D=1024, H=16) on 8 trn2 cores.

Sharding: tensor-parallel over heads x data-parallel over batch.
Core c handles batch b = c // 4, head group g = c % 4 (heads 4g..4g+3).
Host pre-slices/pre-transposes weight+activation shards; each core
returns a partial y (its heads' contribution); host sums groups of 4.

All matmul operands are bf16 (fp32 psum accumulation) — f32r matmuls
lower to fp32_mode=HIGH and trip the PE's 50%-utilization DVFS
throttle, roughly doubling stream time.

The attention loop is software-pipelined: the PE queue stays two
S-units ahead of the PV consumers so exp (Scalar) and causal-mask
multiply (Vector, static 0/1 mask tiles) hide under S/PV streams.
Softmax normalization is per-tile: reciprocal of the PV ones-row,
partition-broadcast on GpSimd, one fused psum-read multiply on Vector.
"""

import os
import sys

for _p in ("/opt/trn_rl_repo", "/root/.axon_site/_ro/trn_rl_repo"):
    if os.path.isdir(_p) and _p not in sys.path:
        sys.path.insert(0, _p)

import ml_dtypes
import numpy as np

import concourse.bass as bass
import concourse.mybir as mybir
import concourse.tile as tile
from concourse import bacc
from concourse.bass_utils import run_bass_kernel_spmd

F32 = mybir.dt.float32
BF16 = mybir.dt.bfloat16

B, T, C = 2, 2048, 1024
NHEAD_TOT = 16
DH = 64
NCORES = 8
NH = 4          # heads per core
NPAIR = 2       # head pairs per core
CK = C // 128   # contraction chunks (8)
TT = 512        # attention t-tile width
NTT = T // TT   # 4
NSCH = T // 128  # s chunks (16)
FQK = 2 * NH * DH  # 512 cols of qkv^T for q+k
FV = NH * DH       # 256 cols for v
LOOKAHEAD = int(os.environ.get("BASS_LOOKAHEAD", "2"))  # S-units in flight ahead of PV


def build_nc(dbg=False):
    nc = bacc.Bacc("TRN2", target_bir_lowering=False, debug=False)

    xT = nc.dram_tensor("xT", [C, T], BF16, kind="ExternalInput")
    wqkvT = nc.dram_tensor("wqkvT", [C, FQK + FV], BF16, kind="ExternalInput")
    woutT = nc.dram_tensor("woutT", [NH * DH, C], BF16, kind="ExternalInput")
    y = nc.dram_tensor("y", [T, C], BF16, kind="ExternalOutput")
    L_dram = nc.dram_tensor("L_scratch", [2 * NPAIR * NTT, TT], F32)
    R_dram = nc.dram_tensor("R_scratch", [2 * NPAIR * NTT, TT], BF16)

    EXP = mybir.ActivationFunctionType.Exp

    with tile.TileContext(nc) as tc:
        with (
            tc.tile_pool(name="const", bufs=1) as const,
            tc.tile_pool(name="ptp", bufs=4) as ptp,
            tc.tile_pool(name="bcp", bufs=2) as bcp,
            tc.tile_pool(name="rcp", bufs=2) as rcp,
            tc.tile_pool(name="yp", bufs=4) as yp,
            tc.tile_pool(name="psA", bufs=2, space="PSUM") as psA,
            tc.tile_pool(name="psV", bufs=4, space="PSUM") as psV,
        ):
            # ---- persistent SBUF ----
            xT_sb = const.tile([128, CK, T], BF16)          # x^T  (c-major)
            wqkvT_sb = const.tile([128, CK, FQK + FV], BF16)  # W_qkv^T cols [q(4x64)|k(4x64)|v(4x64)]
            woutT_sb = const.tile([128, NPAIR, C], BF16)    # W_out^T rows per head pair
            qkT_sb = const.tile([128, 4, T], BF16)          # [qPair0|qPair1|kPair0|kPair1] x T
            v_sb = const.tile([128, NSCH, NH, DH + 1], BF16)  # V (s-major) + ones column
            oT_sb = const.tile([128, NPAIR, T], BF16)       # unnormalized O^T, pair-stacked

            # weights first, then x in t-slice-major order so the first q/k
            # groups (tt=0) start after ~1/4 of the x bytes have landed
            for ci in range(CK):
                nc.sync.dma_start(wqkvT_sb[:, ci, :], wqkvT[ci * 128:(ci + 1) * 128, :])
            for tt in range(NTT):
                for ci in range(CK):
                    nc.sync.dma_start(
                        xT_sb[:, ci, tt * TT:(tt + 1) * TT],
                        xT[ci * 128:(ci + 1) * 128, tt * TT:(tt + 1) * TT])
            for pr in range(NPAIR):
                nc.sync.dma_start(woutT_sb[:, pr, :], woutT[pr * 128:(pr + 1) * 128, :])
            # 1.0 bit pattern for the bf16 ones column
            nc.vector.memset(v_sb[:, :, :, DH:DH + 1].bitcast(mybir.dt.uint16),
                             0x3F80)

            # ---- QKV projection ----
            # q^T/k^T: psum[f128, t512] = sum_c wqkvT[c, f].T @ xT[c, t]
            # tt outer: matches the x DMA arrival order
            for tt in range(NTT):
                for ft in range(4):
                    ps = psA.tile([128, 2, TT], F32)
                    for ci in range(CK):
                        nc.tensor.matmul(
                            ps[:, 0, :],
                            wqkvT_sb[:, ci, ft * 128:(ft + 1) * 128],
                            xT_sb[:, ci, tt * TT:(tt + 1) * TT],
                            start=(ci == 0), stop=(ci == CK - 1),
                        )
                    nc.vector.tensor_copy(qkT_sb[:, ft, tt * TT:(tt + 1) * TT], ps[:, 0, :])
            # v natural: psum[t128, f256] = xT[c, t].T @ wqkvT[c, v]
            for si in range(NSCH):
                ps = psA.tile([128, 2, TT], F32)
                for ci in range(CK):
                    nc.tensor.matmul(
                        ps[:, 0, 0:FV],
                        xT_sb[:, ci, si * 128:(si + 1) * 128],
                        wqkvT_sb[:, ci, FQK:FQK + FV],
                        start=(ci == 0), stop=(ci == CK - 1),
                    )
                nc.vector.tensor_copy(
                    v_sb[:, si, :, 0:DH],
                    ps[:, 0, 0:FV].rearrange("p (h d) -> p h d", h=NH),
                )

            # ---- attention (S^T orientation), per head pair ----
            # software-pipelined: PE queue runs S(u+2) before PV(u) so the
            # exp+mask latency hides under two units of PE stream time.
            for pr in range(NPAIR):
                for tt in range(NTT):
                    n_ss = 4 * (tt + 1)  # causal: s-chunks 0 .. 4*tt+3
                    units = [(sq, hi) for sq in range(n_ss // 2) for hi in range(2)]
                    pv = [psV.tile([DH + 1, TT], F32, tag="pv", name=f"pv{pr}_{tt}_{k}")
                          for k in range(2)]
                    pts = {}

                    def emit_S(u, pr=pr, tt=tt, pts=pts):
                        sq, hi = units[u]
                        ps = psA.tile([128, 2, TT], F32)
                        for i in range(2):
                            ss = 2 * sq + i
                            nc.tensor.matmul(
                                ps[:, i, :],
                                qkT_sb[hi * 64:(hi + 1) * 64, 2 + pr, ss * 128:(ss + 1) * 128],
                                qkT_sb[hi * 64:(hi + 1) * 64, pr, tt * TT:(tt + 1) * TT],
                            )
                        pt = ptp.tile([128, 2, TT], BF16)
                        nc.scalar.activation(pt, ps, EXP, scale=0.125)
                        if sq >= 2 * tt:  # diagonal quad: zero where s > t
                            nc.gpsimd.affine_select(
                                out=pt, in_=pt,
                                compare_op=mybir.AluOpType.is_ge,
                                fill=0.0,
                                base=tt * TT - 2 * sq * 128,
                                channel_multiplier=-1,
                                pattern=[[-128, 2], [1, TT]],
                            )
                        pts[u] = pt

                    def emit_PV(u, pr=pr, tt=tt, n_ss=n_ss, pv=pv, pts=pts):
                        sq, hi = units[u]
                        h = pr * 2 + hi
                        pt = pts.pop(u)
                        for i in range(2):
                            ss = 2 * sq + i
                            nc.tensor.matmul(
                                pv[hi],
                                v_sb[:, ss, h, :],
                                pt[:, i, :],
                                start=(ss == 0), stop=(ss == n_ss - 1),
                            )

                    n_units = len(units)
                    for u in range(min(LOOKAHEAD, n_units)):
                        emit_S(u)
                    for u in range(n_units):
                        if u + LOOKAHEAD < n_units:
                            emit_S(u + LOOKAHEAD)
                        emit_PV(u)

                    # ---- per-tile softmax normalization ----
                    # L rows reshaped to a 16-partition layout via SBUF->SBUF
                    # DMA (single-partition reciprocal is ~6x slower), one
                    # reciprocal, reshaped back, partition-broadcast on GpSimd,
                    # one aligned [128, TT] multiply on Vector. For tt < last
                    # this all hides under the next tile's attention stream.
                    lg = rcp.tile([16, 64], F32, tag="lg", name=f"lg{pr}_{tt}")
                    for hi in range(2):
                        idx = (pr * NTT + tt) * 2 + hi
                        nc.vector.tensor_copy(
                            oT_sb[hi * 64:(hi + 1) * 64, pr, tt * TT:(tt + 1) * TT],
                            pv[hi][0:DH, :],
                        )
                        lrow = rcp.tile([1, TT], F32, tag="lrow", name=f"lrow{idx}")
                        nc.vector.tensor_copy(lrow, pv[hi][DH:DH + 1, :])
                        nc.sync.dma_start(L_dram[idx:idx + 1, :], lrow[0:1, :])
                    base = (pr * NTT + tt) * 2
                    nc.sync.dma_start(
                        lg, L_dram[base:base + 2, :].rearrange("r (s j) -> (r s) j", j=64))
                    with nc.allow_low_precision("1/L in bf16; L is O(1)-scaled"):
                        rc = rcp.tile([16, 64], BF16, tag="rc", name=f"rc{pr}_{tt}")
                        nc.vector.reciprocal(rc, lg)
                    nc.sync.dma_start(
                        R_dram[base:base + 2, :].rearrange("r (s j) -> (r s) j", j=64), rc)
                    rcb = []
                    for hi in range(2):
                        idx = base + hi
                        r1 = rcp.tile([1, TT], BF16, tag="rcb", name=f"rcb{pr}_{tt}_{hi}")
                        nc.sync.dma_start(r1[0:1, :], R_dram[idx:idx + 1, :])
                        rcb.append(r1)
                    rb = bcp.tile([128, TT], BF16, tag="rb", name=f"rb{pr}_{tt}")
                    nc.gpsimd.partition_broadcast(rb, rcb[1][0:1, :], channels=128)
                    nc.gpsimd.partition_broadcast(rb[0:64, :], rcb[0][0:1, :], channels=64)
                    nc.vector.tensor_mul(
                        oT_sb[:, pr, tt * TT:(tt + 1) * TT],
                        oT_sb[:, pr, tt * TT:(tt + 1) * TT],
                        rb,
                    )

            # ---- output projection: y[t, o] = sum_pr oT[d, t].T @ woutT[d, o] ----
            # both psum halves per tq; psum copy-out on the Scalar engine
            # (idle in this phase), one DMA per 128-row block of y.
            for tq in range(T // 128):
                ps = psA.tile([128, 2, TT], F32)
                for ot in range(C // TT):
                    for pr in range(NPAIR):
                        nc.tensor.matmul(
                            ps[:, ot, :],
                            oT_sb[:, pr, tq * 128:(tq + 1) * 128],
                            woutT_sb[:, pr, ot * TT:(ot + 1) * TT],
                            start=(pr == 0), stop=(pr == NPAIR - 1),
                        )
                    yt = yp.tile([128, TT], BF16)
                    if ot == 0:
                        nc.scalar.copy(yt, ps[:, ot, :])
                    else:
                        nc.vector.tensor_copy(yt, ps[:, ot, :])
                    nc.sync.dma_start(
                        y[tq * 128:(tq + 1) * 128, ot * TT:(ot + 1) * TT], yt)

    nc.compile()
    return nc


_NC_CACHE = None


def _get_nc():
    global _NC_CACHE
    if _NC_CACHE is None:
        _NC_CACHE = build_nc()
    return _NC_CACHE


def make_in_maps(x, W_qkv, W_out):
    x = np.ascontiguousarray(np.asarray(x, dtype=np.float32))
    W_qkv = np.ascontiguousarray(np.asarray(W_qkv, dtype=np.float32))
    W_out = np.ascontiguousarray(np.asarray(W_out, dtype=np.float32))
    bf16 = ml_dtypes.bfloat16
    xT = [np.ascontiguousarray(x[b].T.astype(bf16)) for b in range(B)]
    in_maps = []
    for c in range(NCORES):
        b, g = c // 4, c % 4
        rq = W_qkv[g * 256:(g + 1) * 256]            # q rows, heads 4g..4g+3
        rk = W_qkv[C + g * 256:C + (g + 1) * 256]    # k rows
        rv = W_qkv[2 * C + g * 256:2 * C + (g + 1) * 256]  # v rows
        wqkvT = np.ascontiguousarray(
            np.concatenate([rq, rk, rv], axis=0).T.astype(bf16))
        woutT = np.ascontiguousarray(
            W_out[:, g * 256:(g + 1) * 256].T.astype(bf16))
        in_maps.append({"xT": xT[b], "wqkvT": wqkvT, "woutT": woutT})
    return in_maps


def kernel(x, W_qkv, W_out):
    nc = _get_nc()
    in_maps = make_in_maps(x, W_qkv, W_out)
    res = run_bass_kernel_spmd(nc, in_maps, core_ids=list(range(NCORES)))
    kernel.last_results = res
    y = np.zeros((B, T, C), dtype=np.float32)
    for c in range(NCORES):
        y[c // 4] += res.results[c]["y"].astype(np.float32)
    return y


# revision 37
# speedup vs baseline: 1.4249x; 1.0249x over previous
"""Causal self-attention (B=2, T=2048, D=1024, H=16) on 8 trn2 cores.

Sharding: tensor-parallel over heads x data-parallel over batch.
Core c handles batch b = c // 4, head group g = c % 4 (heads 4g..4g+3).
Host pre-slices/pre-transposes weight+activation shards; each core
returns a partial y (its heads' contribution); host sums groups of 4.

All matmul operands are bf16 (fp32 psum accumulation) — f32r matmuls
lower to fp32_mode=HIGH and trip the PE's 50%-utilization DVFS
throttle, roughly doubling stream time.

The attention loop is software-pipelined: the PE queue stays two
S-units ahead of the PV consumers so exp (Scalar) and causal-mask
multiply (Vector, static 0/1 mask tiles) hide under S/PV streams.
Softmax normalization is per-tile: reciprocal of the PV ones-row,
partition-broadcast on GpSimd, one fused psum-read multiply on Vector.
"""

import os
import sys

for _p in ("/opt/trn_rl_repo", "/root/.axon_site/_ro/trn_rl_repo"):
    if os.path.isdir(_p) and _p not in sys.path:
        sys.path.insert(0, _p)

import ml_dtypes
import numpy as np

import concourse.bass as bass
import concourse.mybir as mybir
import concourse.tile as tile
from concourse import bacc
from concourse.bass_utils import run_bass_kernel_spmd

F32 = mybir.dt.float32
BF16 = mybir.dt.bfloat16

B, T, C = 2, 2048, 1024
NHEAD_TOT = 16
DH = 64
NCORES = 8
NH = 4          # heads per core
NPAIR = 2       # head pairs per core
CK = C // 128   # contraction chunks (8)
TT = 512        # attention t-tile width
NTT = T // TT   # 4
NSCH = T // 128  # s chunks (16)
FQK = 2 * NH * DH  # 512 cols of qkv^T for q+k
FV = NH * DH       # 256 cols for v
LOOKAHEAD = int(os.environ.get("BASS_LOOKAHEAD", "2"))  # S-units in flight ahead of PV


def build_nc(dbg=False):
    nc = bacc.Bacc("TRN2", target_bir_lowering=False, debug=False)

    xT = nc.dram_tensor("xT", [C, T], BF16, kind="ExternalInput")
    wqkvT = nc.dram_tensor("wqkvT", [C, FQK + FV], BF16, kind="ExternalInput")
    woutT = nc.dram_tensor("woutT", [NH * DH, C], BF16, kind="ExternalInput")
    y = nc.dram_tensor("y", [T, C], BF16, kind="ExternalOutput")
    L_dram = nc.dram_tensor("L_scratch", [2 * NPAIR * NTT, TT], F32)
    R_dram = nc.dram_tensor("R_scratch", [2 * NPAIR * NTT, TT], BF16)

    EXP = mybir.ActivationFunctionType.Exp

    with tile.TileContext(nc) as tc:
        with (
            tc.tile_pool(name="const", bufs=1) as const,
            tc.tile_pool(name="ptp", bufs=4) as ptp,
            tc.tile_pool(name="bcp", bufs=2) as bcp,
            tc.tile_pool(name="rcp", bufs=2) as rcp,
            tc.tile_pool(name="yp", bufs=4) as yp,
            tc.tile_pool(name="psA", bufs=2, space="PSUM") as psA,
            tc.tile_pool(name="psV", bufs=4, space="PSUM") as psV,
        ):
            # ---- persistent SBUF ----
            xT_sb = const.tile([128, CK, T], BF16)          # x^T  (c-major)
            wqkvT_sb = const.tile([128, CK, FQK + FV], BF16)  # W_qkv^T cols [q(4x64)|k(4x64)|v(4x64)]
            woutT_sb = const.tile([128, NPAIR, C], BF16)    # W_out^T rows per head pair
            qkT_sb = const.tile([128, 4, T], BF16)          # [qPair0|qPair1|kPair0|kPair1] x T
            v_sb = const.tile([128, NSCH, NH, DH + 1], BF16)  # V (s-major) + ones column
            oT_sb = const.tile([128, NPAIR, T], BF16)       # unnormalized O^T, pair-stacked

            # weights first, then x in t-slice-major order so the first q/k
            # groups (tt=0) start after ~1/4 of the x bytes have landed
            for ci in range(CK):
                nc.sync.dma_start(wqkvT_sb[:, ci, :], wqkvT[ci * 128:(ci + 1) * 128, :])
            for tt in range(NTT):
                for ci in range(CK):
                    nc.sync.dma_start(
                        xT_sb[:, ci, tt * TT:(tt + 1) * TT],
                        xT[ci * 128:(ci + 1) * 128, tt * TT:(tt + 1) * TT])
            for pr in range(NPAIR):
                nc.sync.dma_start(woutT_sb[:, pr, :], woutT[pr * 128:(pr + 1) * 128, :])
            # 1.0 bit pattern for the bf16 ones column
            nc.vector.memset(v_sb[:, :, :, DH:DH + 1].bitcast(mybir.dt.uint16),
                             0x3F80)

            # ---- QKV projection ----
            # q^T/k^T: psum[f128, t512] = sum_c wqkvT[c, f].T @ xT[c, t]
            # tt outer: matches the x DMA arrival order
            for tt in range(NTT):
                for ft in range(4):
                    ps = psA.tile([128, 2, TT], F32)
                    for ci in range(CK):
                        nc.tensor.matmul(
                            ps[:, 0, :],
                            wqkvT_sb[:, ci, ft * 128:(ft + 1) * 128],
                            xT_sb[:, ci, tt * TT:(tt + 1) * TT],
                            start=(ci == 0), stop=(ci == CK - 1),
                        )
                    nc.vector.tensor_copy(qkT_sb[:, ft, tt * TT:(tt + 1) * TT], ps[:, 0, :])
            # v natural: psum[t128, f256] = xT[c, t].T @ wqkvT[c, v]
            for si in range(NSCH):
                ps = psA.tile([128, 2, TT], F32)
                for ci in range(CK):
                    nc.tensor.matmul(
                        ps[:, 0, 0:FV],
                        xT_sb[:, ci, si * 128:(si + 1) * 128],
                        wqkvT_sb[:, ci, FQK:FQK + FV],
                        start=(ci == 0), stop=(ci == CK - 1),
                    )
                nc.vector.tensor_copy(
                    v_sb[:, si, :, 0:DH],
                    ps[:, 0, 0:FV].rearrange("p (h d) -> p h d", h=NH),
                )

            # ---- attention (S^T orientation), per head pair ----
            # software-pipelined: PE queue runs S(u+2) before PV(u) so the
            # exp+mask latency hides under two units of PE stream time.
            for pr in range(NPAIR):
                for tt in range(NTT):
                    n_ss = 4 * (tt + 1)  # causal: s-chunks 0 .. 4*tt+3
                    units = [(sq, hi) for sq in range(n_ss // 2) for hi in range(2)]
                    pv = [psV.tile([128, TT], F32, tag="pv",
                                   name=f"pv{pr}_{tt}_{k}")[0:DH + 1, :]
                          for k in range(2)]
                    pts = {}

                    def emit_S(u, pr=pr, tt=tt, pts=pts):
                        sq, hi = units[u]
                        ps = psA.tile([128, 2, TT], F32)
                        for i in range(2):
                            ss = 2 * sq + i
                            nc.tensor.matmul(
                                ps[:, i, :],
                                qkT_sb[hi * 64:(hi + 1) * 64, 2 + pr, ss * 128:(ss + 1) * 128],
                                qkT_sb[hi * 64:(hi + 1) * 64, pr, tt * TT:(tt + 1) * TT],
                            )
                        pt = ptp.tile([128, 2, TT], BF16)
                        nc.scalar.activation(pt, ps, EXP, scale=0.125)
                        if sq >= 2 * tt:  # diagonal quad: zero where s > t
                            nc.gpsimd.affine_select(
                                out=pt, in_=pt,
                                compare_op=mybir.AluOpType.is_ge,
                                fill=0.0,
                                base=tt * TT - 2 * sq * 128,
                                channel_multiplier=-1,
                                pattern=[[-128, 2], [1, TT]],
                            )
                        pts[u] = pt

                    def emit_PV(u, pr=pr, tt=tt, n_ss=n_ss, pv=pv, pts=pts):
                        sq, hi = units[u]
                        h = pr * 2 + hi
                        pt = pts.pop(u)
                        for i in range(2):
                            ss = 2 * sq + i
                            nc.tensor.matmul(
                                pv[hi],
                                v_sb[:, ss, h, :],
                                pt[:, i, :],
                                start=(ss == 0), stop=(ss == n_ss - 1),
                            )

                    n_units = len(units)
                    for u in range(min(LOOKAHEAD, n_units)):
                        emit_S(u)
                    for u in range(n_units):
                        if u + LOOKAHEAD < n_units:
                            emit_S(u + LOOKAHEAD)
                        emit_PV(u)

                    # ---- per-tile softmax normalization ----
                    # L rows reshaped to a 16-partition layout via SBUF->SBUF
                    # DMA (single-partition reciprocal is ~6x slower), one
                    # reciprocal, reshaped back, partition-broadcast on GpSimd,
                    # one aligned [128, TT] multiply on Vector. For tt < last
                    # this all hides under the next tile's attention stream.
                    lg = rcp.tile([16, 64], F32, tag="lg", name=f"lg{pr}_{tt}")
                    for hi in range(2):
                        idx = (pr * NTT + tt) * 2 + hi
                        nc.vector.tensor_copy(
                            oT_sb[hi * 64:(hi + 1) * 64, pr, tt * TT:(tt + 1) * TT],
                            pv[hi][0:DH, :],
                        )
                        lrow = rcp.tile([1, TT], F32, tag="lrow", name=f"lrow{idx}")
                        nc.vector.tensor_copy(lrow, pv[hi][DH:DH + 1, :])
                        nc.sync.dma_start(L_dram[idx:idx + 1, :], lrow[0:1, :])
                    base = (pr * NTT + tt) * 2
                    nc.sync.dma_start(
                        lg, L_dram[base:base + 2, :].rearrange("r (s j) -> (r s) j", j=64))
                    with nc.allow_low_precision("1/L in bf16; L is O(1)-scaled"):
                        rc = rcp.tile([16, 64], BF16, tag="rc", name=f"rc{pr}_{tt}")
                        nc.vector.reciprocal(rc, lg)
                    nc.sync.dma_start(
                        R_dram[base:base + 2, :].rearrange("r (s j) -> (r s) j", j=64), rc)
                    rcb = []
                    for hi in range(2):
                        idx = base + hi
                        r1 = rcp.tile([1, TT], BF16, tag="rcb", name=f"rcb{pr}_{tt}_{hi}")
                        nc.sync.dma_start(r1[0:1, :], R_dram[idx:idx + 1, :])
                        rcb.append(r1)
                    rb = bcp.tile([128, TT], BF16, tag="rb", name=f"rb{pr}_{tt}")
                    nc.gpsimd.partition_broadcast(rb, rcb[1][0:1, :], channels=128)
                    nc.gpsimd.partition_broadcast(rb[0:64, :], rcb[0][0:1, :], channels=64)
                    nc.vector.tensor_mul(
                        oT_sb[:, pr, tt * TT:(tt + 1) * TT],
                        oT_sb[:, pr, tt * TT:(tt + 1) * TT],
                        rb,
                    )

            # ---- output projection: y[t, o] = sum_pr oT[d, t].T @ woutT[d, o] ----
            # both psum halves per tq; psum copy-out on the Scalar engine
            # (idle in this phase), one DMA per 128-row block of y.
            for tq in range(T // 128):
                for ot in range(C // TT):
                    ps = psV.tile([128, TT], F32, tag="pv", name=f"pj{tq}_{ot}")
                    for pr in range(NPAIR):
                        nc.tensor.matmul(
                            ps,
                            oT_sb[:, pr, tq * 128:(tq + 1) * 128],
                            woutT_sb[:, pr, ot * TT:(ot + 1) * TT],
                            start=(pr == 0), stop=(pr == NPAIR - 1),
                        )
                    yt = yp.tile([128, TT], BF16)
                    if ot == 0:
                        nc.scalar.copy(yt, ps)
                    else:
                        nc.vector.tensor_copy(yt, ps)
                    nc.sync.dma_start(
                        y[tq * 128:(tq + 1) * 128, ot * TT:(ot + 1) * TT], yt)

    nc.compile()
    return nc


_NC_CACHE = None


def _get_nc():
    global _NC_CACHE
    if _NC_CACHE is None:
        _NC_CACHE = build_nc()
    return _NC_CACHE


def make_in_maps(x, W_qkv, W_out):
    x = np.ascontiguousarray(np.asarray(x, dtype=np.float32))
    W_qkv = np.ascontiguousarray(np.asarray(W_qkv, dtype=np.float32))
    W_out = np.ascontiguousarray(np.asarray(W_out, dtype=np.float32))
    bf16 = ml_dtypes.bfloat16
    xT = [np.ascontiguousarray(x[b].T.astype(bf16)) for b in range(B)]
    in_maps = []
    for c in range(NCORES):
        b, g = c // 4, c % 4
        rq = W_qkv[g * 256:(g + 1) * 256]            # q rows, heads 4g..4g+3
        rk = W_qkv[C + g * 256:C + (g + 1) * 256]    # k rows
        rv = W_qkv[2 * C + g * 256:2 * C + (g + 1) * 256]  # v rows
        wqkvT = np.ascontiguousarray(
            np.concatenate([rq, rk, rv], axis=0).T.astype(bf16))
        woutT = np.ascontiguousarray(
            W_out[:, g * 256:(g + 1) * 256].T.astype(bf16))
        in_maps.append({"xT": xT[b], "wqkvT": wqkvT, "woutT": woutT})
    return in_maps


def kernel(x, W_qkv, W_out):
    nc = _get_nc()
    in_maps = make_in_maps(x, W_qkv, W_out)
    res = run_bass_kernel_spmd(nc, in_maps, core_ids=list(range(NCORES)))
    kernel.last_results = res
    y = np.zeros((B, T, C), dtype=np.float32)
    for c in range(NCORES):
        y[c // 4] += res.results[c]["y"].astype(np.float32)
    return y
